# revision 10
# baseline (speedup 1.0000x reference)
"""Haar DWT (db1, zero-padded left/top, stride 2) on 8 Trainium2 NeuronCores.

Math (per image, per output index h, w in [0, 512)):
  with A=x[2h-1,2w-1], B=x[2h-1,2w], C=x[2h,2w-1], D=x[2h,2w]  (x[-1,*]=x[*,-1]=0)
  LL = .5(A+B+C+D)   LH = .5(C+D-A-B)   HL = .5(B-A+D-C)   HH = .5(A-B-C+D)

Kernel strategy per core (6 of the 48 (b,c) images, pure data parallel):
  - x rows tiled into 128-row SBUF tiles; tile 0 holds rows 0..127, tile t>0
    holds rows 128t-1..128t+126, so vertical Haar pairs are adjacent
    partition pairs inside one tile.
  - Vertical stage on the TensorEngine: a fp32 128x128 weight matrix holds
    both vertical filters (lo -> psum partitions 0:64, hi -> 64:128) with the
    full 0.5 scale folded in; tile 0 uses a variant whose first output row
    reads only x row 0 (row -1 is zero).  psum[p, j] = vertical result at
    x column j.
  - PSUM is evacuated by a ScalarE copy; the horizontal stage is then two
    VectorE ops per tile: pair-sum of columns (2w-1, 2w) -> LL/LH bands,
    pair-diff -> HL/HH.  Column 0 of each band is the vertical result at
    x column 0 (x column -1 is zero) - two tiny ScalarE copies.
  - All four bands land in ONE dram tensor out4[4, n_img, 512, 512] so each
    per-image band-pair tile drains with a single 128-partition 2 MiB DMA.
"""

import numpy as np

_H = 1024
_W = 1024
_N_CORES = 8


def _build_bass(n_img: int, H: int, W: int):
    import concourse.bass as bass
    import concourse.tile as tile
    from concourse import bacc, mybir

    f32 = mybir.dt.float32
    Ho, Wo = H // 2, W // 2
    n_t = H // 128  # 128-row tiles per image

    # Bacc (not raw Bass): its compile() legalizes sync waits — TRN2 allows
    # at most 1 wait per instruction and walrus rejects the raw Tile output
    nc = bacc.Bacc("TRN2", target_bir_lowering=False, debug=False)
    x = nc.declare_dram_parameter("x", [n_img, H, W], f32, isOutput=False)
    wv = nc.declare_dram_parameter("wv", [2, 128, 128], f32, isOutput=False)
    # [P/M, img, row-tile, lo/hi, p, w]: (lo/hi, p) merges into one
    # 128-partition DMA dim, so each band-pair tile drains in ONE 3-dim DMA
    out2 = nc.declare_dram_parameter(
        "out2", [2, n_img, n_t, 2, 64, Wo], f32, isOutput=True
    )

    # matmul width chunks (fp32 moving operand max 512)
    chunks = [(c, min(512, W - c)) for c in range(0, W, 512)]

    n_mid = 4
    with tile.TileContext(nc) as tc:
        with (
            tc.tile_pool(name="wpool", bufs=1) as wpool,
            tc.tile_pool(name="xin", bufs=2) as xin,
            tc.tile_pool(name="ps", bufs=4, space=bass.MemorySpace.PSUM) as pspool,
            tc.tile_pool(name="mid", bufs=1) as midpool,
            tc.tile_pool(name="band", bufs=2) as bandpool,
        ):
            wt = wpool.tile([128, 256], f32)
            # wv[0] = first-row-tile weights, wv[1] = standard weights
            nc.sync.dma_start(wt[:, 0:128], wv[0])
            nc.sync.dma_start(wt[:, 128:256], wv[1])

            # persistent psum-evac tiles; col 0 is a zero pad (x col -1) so the
            # horizontal stage is one uniform stride-2 pair op, written once
            mids = []
            for k in range(n_mid):
                m = midpool.tile([128, W + 8], f32, tag=f"mid{k}")
                nc.gpsimd.memset(m[:, 0:1], 0.0)
                mids.append(m)

            for i in range(n_img):
                bp = bandpool.tile([128, n_t * Wo], f32, tag="bp")
                bm = bandpool.tile([128, n_t * Wo], f32, tag="bm")
                # row tile 0 = x rows 0..127; row tile t>0 = rows 128t-1..+126
                xt = xin.tile([128, n_t * W], f32)
                nc.sync.dma_start(xt[:, 0:W], x[i, 0:128, :])
                nc.sync.dma_start(
                    xt[:, W : n_t * W].rearrange("p (t w) -> p t w", t=n_t - 1),
                    x[i, 127 : 128 * (n_t - 1) + 127, :].rearrange(
                        "(t p) w -> p t w", p=128
                    ),
                )
                for t in range(n_t):
                    w_ap = wt[:, 0:128] if t == 0 else wt[:, 128:256]
                    ps = pspool.tile([128, W], f32)
                    for c, cw in chunks:
                        nc.tensor.matmul(
                            ps[:, c : c + cw],
                            w_ap,
                            xt[:, t * W + c : t * W + c + cw],
                            start=True,
                            stop=True,
                        )

                    # evacuate PSUM next to the zero pad: mid col j+1 = x col j
                    mid = mids[(i * n_t + t) % n_mid]
                    nc.scalar.copy(mid[:, 1 : W + 1], ps[:])

                    # horizontal pairs: out w <- mid cols (2w, 2w+1)
                    o = t * Wo
                    nc.vector.tensor_add(
                        bp[:, o : o + Wo], mid[:, 0:W:2], mid[:, 1:W:2]
                    )
                    nc.vector.tensor_sub(
                        bm[:, o : o + Wo], mid[:, 1:W:2], mid[:, 0:W:2]
                    )

                    # drain half-image chunks: partitions 0:64 = lo band rows,
                    # 64:128 = hi; finishing earlier shortens the kernel tail
                    if t % (n_t // 2) == n_t // 2 - 1:
                        h0 = t + 1 - n_t // 2
                        sl = slice(h0 * Wo, (t + 1) * Wo)
                        nc.sync.dma_start(
                            out2[0, i, h0 : t + 1].rearrange("t b p w -> (b p) t w"),
                            bp[:, sl].rearrange("pp (t w) -> pp t w", t=n_t // 2),
                        )
                        nc.sync.dma_start(
                            out2[1, i, h0 : t + 1].rearrange("t b p w -> (b p) t w"),
                            bm[:, sl].rearrange("pp (t w) -> pp t w", t=n_t // 2),
                        )
    # run Bacc's legalization passes (walrus needs <=1 sync wait per inst);
    # run_bass_via_pjrt serializes the module as-is
    nc.finalize()
    return nc


def _wv_matrices(h0_v, h1_v, hscale):
    """[2,128,128]: [0] = first-row-tile weights (pairs shifted, row -1 is
    zero), [1] = standard weights for tiles starting at row 128t-1."""
    wv = np.zeros((2, 128, 128), np.float32)
    j = np.arange(64)
    # standard: tile partition k holds x row 128t-1+k; out j pairs k=(2j, 2j+1)
    wv[1, 2 * j, j] = hscale * h0_v[0]
    wv[1, 2 * j + 1, j] = hscale * h0_v[1]
    wv[1, 2 * j, 64 + j] = hscale * h1_v[0]
    wv[1, 2 * j + 1, 64 + j] = hscale * h1_v[1]
    # tile 0: partition k holds x row k; out j pairs k=(2j-1, 2j)
    wv[0, 0, 0] = hscale * h0_v[1]
    wv[0, 0, 64] = hscale * h1_v[1]
    j = np.arange(1, 64)
    wv[0, 2 * j - 1, j] = hscale * h0_v[0]
    wv[0, 2 * j, j] = hscale * h0_v[1]
    wv[0, 2 * j - 1, 64 + j] = hscale * h1_v[0]
    wv[0, 2 * j, 64 + j] = hscale * h1_v[1]
    return wv


def _ensure_ntff_hook():
    """Provide antenv.axon_hooks + register the ctypes NTFF profile hook
    (the agent image's antenv lacks axon_hooks, so tracing would crash)."""
    import contextlib
    import ctypes
    import sys
    import types

    if "antenv.axon_hooks" not in sys.modules:
        mod = types.ModuleType("antenv.axon_hooks")
        state = {"hook": None}
        mod.set_axon_ntff_profile_hook = lambda h: state.__setitem__("hook", h)
        mod.get_axon_ntff_profile_hook = lambda: state["hook"]
        sys.modules["antenv.axon_hooks"] = mod
        try:
            import antenv

            antenv.axon_hooks = mod
        except ImportError:
            pass
    hooks = sys.modules["antenv.axon_hooks"]
    if hooks.get_axon_ntff_profile_hook() is not None:
        return

    lib = ctypes.CDLL("/opt/axon/libaxon_pjrt.so")
    if not hasattr(lib, "axon_start_nrt_profile"):
        return
    lib.axon_start_nrt_profile.argtypes = [
        ctypes.POINTER(ctypes.c_int64),
        ctypes.c_size_t,
    ]
    lib.axon_start_nrt_profile.restype = ctypes.c_int64
    lib.axon_stop_nrt_profile.argtypes = [ctypes.c_char_p]
    lib.axon_stop_nrt_profile.restype = ctypes.c_int64

    @contextlib.contextmanager
    def _hook(output_dir, device_ids):
        import jax

        jax.devices()
        if device_ids:
            ids = (ctypes.c_int64 * len(device_ids))(*device_ids)
            rc = lib.axon_start_nrt_profile(ids, len(device_ids))
        else:
            rc = lib.axon_start_nrt_profile(None, 0)
        if rc != 0:
            raise RuntimeError(f"axon_start_nrt_profile rc={rc}")
        try:
            yield
        finally:
            n = lib.axon_stop_nrt_profile(str(output_dir).encode())
            print(f"profile: {n} file(s) written to {output_dir}", file=sys.stderr)

    hooks.set_axon_ntff_profile_hook(_hook)

    # the post-run artifact upload needs bucket creds we don't have
    import concourse.bass_utils as bu

    bu.upload_artifacts = lambda tmpdir: f"local://{tmpdir}"


def _run(x, wv, trace=False, tmpdir=None):
    """x: (48, 1024, 1024) f32. Returns dict band -> (48, 512, 512), results."""
    import sys

    if "/opt/trn_rl_repo" not in sys.path:
        sys.path.insert(0, "/opt/trn_rl_repo")
    if trace:
        _ensure_ntff_hook()
    from concourse.bass_utils import run_bass_kernel_spmd

    n_total = x.shape[0]
    per = n_total // _N_CORES
    nc = _build_bass(per, _H, _W)
    in_maps = [
        {"x": np.ascontiguousarray(x[c * per : (c + 1) * per]), "wv": wv}
        for c in range(_N_CORES)
    ]
    res = run_bass_kernel_spmd(
        nc, in_maps, list(range(_N_CORES)), trace=trace, tmpdir=tmpdir
    )
    # out2: (2, per, n_t, 2, 64, Wo); h = 64*t + p
    parts = [res.results[c]["out2"] for c in range(_N_CORES)]
    full = np.concatenate(parts, axis=1)  # (2, 48, n_t, 2, 64, Wo)
    n_total_, nt_, _, _, wo_ = full.shape[1:]
    banded = full.transpose(0, 3, 1, 2, 4, 5).reshape(2, 2, n_total_, nt_ * 64, wo_)
    return {
        "ll": banded[0, 0],
        "lh": banded[0, 1],
        "hl": banded[1, 0],
        "hh": banded[1, 1],
    }, res


def kernel(x, h0_h, h1_h, h0_v, h1_v, _trace=False, _tmpdir=None):
    x = np.asarray(x, dtype=np.float32)
    h0_h = np.asarray(h0_h, dtype=np.float32).reshape(-1)
    h1_h = np.asarray(h1_h, dtype=np.float32).reshape(-1)
    h0_v = np.asarray(h0_v, dtype=np.float32).reshape(-1)
    h1_v = np.asarray(h1_v, dtype=np.float32).reshape(-1)

    # the horizontal stage is a plain pair sum/diff; that requires the
    # Haar shape [s, s] / [-s, s] (always the case for this problem)
    hs = float(h0_h[1])
    assert abs(h0_h[0] - hs) < 1e-6 and abs(h1_h[1] - hs) < 1e-6
    assert abs(h1_h[0] + hs) < 1e-6

    B, C, H, W = x.shape
    xf = x.reshape(B * C, H, W)
    wv = _wv_matrices(h0_v, h1_v, hs)
    out, res = _run(xf, wv, trace=_trace, tmpdir=_tmpdir)
    rs = lambda t: np.ascontiguousarray(t.reshape(B, C, H // 2, W // 2))
    result = (rs(out["ll"]), (rs(out["lh"]), rs(out["hl"]), rs(out["hh"])))
    if _trace:
        return result, res
    return result


# revision 11
# speedup vs baseline: 1.2802x; 1.2802x over previous
"""Haar DWT (db1, zero-padded left/top, stride 2) on 8 Trainium2 NeuronCores.

Math (per image, per output index h, w in [0, 512)):
  with A=x[2h-1,2w-1], B=x[2h-1,2w], C=x[2h,2w-1], D=x[2h,2w]  (x[-1,*]=x[*,-1]=0)
  LL = .5(A+B+C+D)   LH = .5(C+D-A-B)   HL = .5(B-A+D-C)   HH = .5(A-B-C+D)

Kernel strategy per core (6 of the 48 (b,c) images, pure data parallel):
  - x rows tiled into 128-row SBUF tiles; tile 0 holds rows 0..127, tile t>0
    holds rows 128t-1..128t+126, so vertical Haar pairs are adjacent
    partition pairs inside one tile.
  - Vertical stage on the TensorEngine: a fp32 128x128 weight matrix holds
    both vertical filters (lo -> psum partitions 0:64, hi -> 64:128) with the
    full 0.5 scale folded in; tile 0 uses a variant whose first output row
    reads only x row 0 (row -1 is zero).  psum[p, j] = vertical result at
    x column j.
  - PSUM is evacuated by a ScalarE copy; the horizontal stage is then two
    VectorE ops per tile: pair-sum of columns (2w-1, 2w) -> LL/LH bands,
    pair-diff -> HL/HH.  Column 0 of each band is the vertical result at
    x column 0 (x column -1 is zero) - two tiny ScalarE copies.
  - All four bands land in ONE dram tensor out4[4, n_img, 512, 512] so each
    per-image band-pair tile drains with a single 128-partition 2 MiB DMA.
"""

import numpy as np

_H = 1024
_W = 1024
_N_CORES = 8


def _build_bass(n_img: int, H: int, W: int):
    import concourse.bass as bass
    import concourse.tile as tile
    from concourse import bacc, mybir

    f32 = mybir.dt.float32
    Ho, Wo = H // 2, W // 2
    n_t = H // 128  # 128-row tiles per image

    # Bacc (not raw Bass): its compile() legalizes sync waits — TRN2 allows
    # at most 1 wait per instruction and walrus rejects the raw Tile output
    nc = bacc.Bacc("TRN2", target_bir_lowering=False, debug=False)
    x = nc.declare_dram_parameter("x", [n_img, H, W], f32, isOutput=False)
    wv = nc.declare_dram_parameter("wv", [2, 128, 128], f32, isOutput=False)
    # [P/M, img, row-tile, lo/hi, p, w]: (lo/hi, p) merges into one
    # 128-partition DMA dim, so each band-pair tile drains in ONE 3-dim DMA
    out2 = nc.declare_dram_parameter(
        "out2", [2, n_img, n_t, 2, 64, Wo], f32, isOutput=True
    )

    # matmul width chunks (fp32 moving operand max 512)
    chunks = [(c, min(512, W - c)) for c in range(0, W, 512)]

    n_mid = 4
    # input DMA chunks: row-tile 0 alone (small, unblocks the first matmuls
    # fast), then the rest in two pieces for finer-grained prefetch
    if n_t > 2:
        in_chunks = [(1, n_t // 2), (n_t // 2, n_t)]
    else:
        in_chunks = [(1, n_t)]
    with tile.TileContext(nc) as tc:
        with (
            tc.tile_pool(name="wpool", bufs=1) as wpool,
            tc.tile_pool(name="xin", bufs=3) as xin,
            tc.tile_pool(name="ps", bufs=4, space=bass.MemorySpace.PSUM) as pspool,
            tc.tile_pool(name="mid", bufs=1) as midpool,
            tc.tile_pool(name="band", bufs=2) as bandpool,
        ):
            wt = wpool.tile([128, 256], f32)
            # wv[0] = first-row-tile weights, wv[1] = standard weights
            nc.sync.dma_start(wt[:, 0:128], wv[0])
            nc.sync.dma_start(wt[:, 128:256], wv[1])

            # persistent psum-evac tiles; col 1 is a zero pad (x col -1), data
            # starts at col 2 (8B-aligned so the ScalarE copy keeps 2x mode)
            mids = []
            for k in range(n_mid):
                m = midpool.tile([128, W + 8], f32, tag=f"mid{k}")
                nc.gpsimd.memset(m[:, 0:2], 0.0)
                mids.append(m)

            for i in range(n_img):
                bp = bandpool.tile([128, n_t * Wo], f32, tag="bp")
                bm = bandpool.tile([128, n_t * Wo], f32, tag="bm")
                # row tile 0 = x rows 0..127; row tile t>0 = rows 128t-1..+126
                xt = xin.tile([128, n_t * W], f32)
                nc.sync.dma_start(xt[:, 0:W], x[i, 0:128, :])
                for ta, tb in in_chunks:
                    nc.sync.dma_start(
                        xt[:, ta * W : tb * W].rearrange("p (t w) -> p t w", t=tb - ta),
                        x[i, 128 * ta - 1 : 128 * tb - 1, :].rearrange(
                            "(t p) w -> p t w", p=128
                        ),
                    )
                for t in range(n_t):
                    w_ap = wt[:, 0:128] if t == 0 else wt[:, 128:256]
                    ps = pspool.tile([128, W], f32)
                    for c, cw in chunks:
                        nc.tensor.matmul(
                            ps[:, c : c + cw],
                            w_ap,
                            xt[:, t * W + c : t * W + c + cw],
                            start=True,
                            stop=True,
                        )

                    # evacuate PSUM next to the zero pad: mid col j+2 = x col j
                    mid = mids[(i * n_t + t) % n_mid]
                    nc.scalar.copy(mid[:, 2 : W + 2], ps[:])

                    # horizontal pairs: out w <- mid cols (2w+1, 2w+2)
                    o = t * Wo
                    nc.vector.tensor_add(
                        bp[:, o : o + Wo], mid[:, 1 : W + 1 : 2], mid[:, 2 : W + 2 : 2]
                    )
                    nc.vector.tensor_sub(
                        bm[:, o : o + Wo], mid[:, 2 : W + 2 : 2], mid[:, 1 : W + 1 : 2]
                    )

                    # drain half-image chunks on the SWDGE ring: keeping stores
                    # off the HWDGE ring lets next-image loads issue immediately
                    if t % (n_t // 2) == n_t // 2 - 1:
                        h0 = t + 1 - n_t // 2
                        sl = slice(h0 * Wo, (t + 1) * Wo)
                        nc.gpsimd.dma_start(
                            out2[0, i, h0 : t + 1].rearrange("t b p w -> (b p) t w"),
                            bp[:, sl].rearrange("pp (t w) -> pp t w", t=n_t // 2),
                        )
                        nc.gpsimd.dma_start(
                            out2[1, i, h0 : t + 1].rearrange("t b p w -> (b p) t w"),
                            bm[:, sl].rearrange("pp (t w) -> pp t w", t=n_t // 2),
                        )
    # run Bacc's legalization passes (walrus needs <=1 sync wait per inst);
    # run_bass_via_pjrt serializes the module as-is
    nc.finalize()
    return nc


def _wv_matrices(h0_v, h1_v, hscale):
    """[2,128,128]: [0] = first-row-tile weights (pairs shifted, row -1 is
    zero), [1] = standard weights for tiles starting at row 128t-1."""
    wv = np.zeros((2, 128, 128), np.float32)
    j = np.arange(64)
    # standard: tile partition k holds x row 128t-1+k; out j pairs k=(2j, 2j+1)
    wv[1, 2 * j, j] = hscale * h0_v[0]
    wv[1, 2 * j + 1, j] = hscale * h0_v[1]
    wv[1, 2 * j, 64 + j] = hscale * h1_v[0]
    wv[1, 2 * j + 1, 64 + j] = hscale * h1_v[1]
    # tile 0: partition k holds x row k; out j pairs k=(2j-1, 2j)
    wv[0, 0, 0] = hscale * h0_v[1]
    wv[0, 0, 64] = hscale * h1_v[1]
    j = np.arange(1, 64)
    wv[0, 2 * j - 1, j] = hscale * h0_v[0]
    wv[0, 2 * j, j] = hscale * h0_v[1]
    wv[0, 2 * j - 1, 64 + j] = hscale * h1_v[0]
    wv[0, 2 * j, 64 + j] = hscale * h1_v[1]
    return wv


def _ensure_ntff_hook():
    """Provide antenv.axon_hooks + register the ctypes NTFF profile hook
    (the agent image's antenv lacks axon_hooks, so tracing would crash)."""
    import contextlib
    import ctypes
    import sys
    import types

    if "antenv.axon_hooks" not in sys.modules:
        mod = types.ModuleType("antenv.axon_hooks")
        state = {"hook": None}
        mod.set_axon_ntff_profile_hook = lambda h: state.__setitem__("hook", h)
        mod.get_axon_ntff_profile_hook = lambda: state["hook"]
        sys.modules["antenv.axon_hooks"] = mod
        try:
            import antenv

            antenv.axon_hooks = mod
        except ImportError:
            pass
    hooks = sys.modules["antenv.axon_hooks"]
    if hooks.get_axon_ntff_profile_hook() is not None:
        return

    lib = ctypes.CDLL("/opt/axon/libaxon_pjrt.so")
    if not hasattr(lib, "axon_start_nrt_profile"):
        return
    lib.axon_start_nrt_profile.argtypes = [
        ctypes.POINTER(ctypes.c_int64),
        ctypes.c_size_t,
    ]
    lib.axon_start_nrt_profile.restype = ctypes.c_int64
    lib.axon_stop_nrt_profile.argtypes = [ctypes.c_char_p]
    lib.axon_stop_nrt_profile.restype = ctypes.c_int64

    @contextlib.contextmanager
    def _hook(output_dir, device_ids):
        import jax

        jax.devices()
        if device_ids:
            ids = (ctypes.c_int64 * len(device_ids))(*device_ids)
            rc = lib.axon_start_nrt_profile(ids, len(device_ids))
        else:
            rc = lib.axon_start_nrt_profile(None, 0)
        if rc != 0:
            raise RuntimeError(f"axon_start_nrt_profile rc={rc}")
        try:
            yield
        finally:
            n = lib.axon_stop_nrt_profile(str(output_dir).encode())
            print(f"profile: {n} file(s) written to {output_dir}", file=sys.stderr)

    hooks.set_axon_ntff_profile_hook(_hook)

    # the post-run artifact upload needs bucket creds we don't have
    import concourse.bass_utils as bu

    bu.upload_artifacts = lambda tmpdir: f"local://{tmpdir}"


def _run(x, wv, trace=False, tmpdir=None):
    """x: (48, 1024, 1024) f32. Returns dict band -> (48, 512, 512), results."""
    import sys

    if "/opt/trn_rl_repo" not in sys.path:
        sys.path.insert(0, "/opt/trn_rl_repo")
    if trace:
        _ensure_ntff_hook()
    from concourse.bass_utils import run_bass_kernel_spmd

    n_total = x.shape[0]
    per = n_total // _N_CORES
    nc = _build_bass(per, _H, _W)
    in_maps = [
        {"x": np.ascontiguousarray(x[c * per : (c + 1) * per]), "wv": wv}
        for c in range(_N_CORES)
    ]
    res = run_bass_kernel_spmd(
        nc, in_maps, list(range(_N_CORES)), trace=trace, tmpdir=tmpdir
    )
    # out2: (2, per, n_t, 2, 64, Wo); h = 64*t + p
    parts = [res.results[c]["out2"] for c in range(_N_CORES)]
    full = np.concatenate(parts, axis=1)  # (2, 48, n_t, 2, 64, Wo)
    n_total_, nt_, _, _, wo_ = full.shape[1:]
    banded = full.transpose(0, 3, 1, 2, 4, 5).reshape(2, 2, n_total_, nt_ * 64, wo_)
    return {
        "ll": banded[0, 0],
        "lh": banded[0, 1],
        "hl": banded[1, 0],
        "hh": banded[1, 1],
    }, res


def kernel(x, h0_h, h1_h, h0_v, h1_v, _trace=False, _tmpdir=None):
    x = np.asarray(x, dtype=np.float32)
    h0_h = np.asarray(h0_h, dtype=np.float32).reshape(-1)
    h1_h = np.asarray(h1_h, dtype=np.float32).reshape(-1)
    h0_v = np.asarray(h0_v, dtype=np.float32).reshape(-1)
    h1_v = np.asarray(h1_v, dtype=np.float32).reshape(-1)

    # the horizontal stage is a plain pair sum/diff; that requires the
    # Haar shape [s, s] / [-s, s] (always the case for this problem)
    hs = float(h0_h[1])
    assert abs(h0_h[0] - hs) < 1e-6 and abs(h1_h[1] - hs) < 1e-6
    assert abs(h1_h[0] + hs) < 1e-6

    B, C, H, W = x.shape
    xf = x.reshape(B * C, H, W)
    wv = _wv_matrices(h0_v, h1_v, hs)
    out, res = _run(xf, wv, trace=_trace, tmpdir=_tmpdir)
    rs = lambda t: np.ascontiguousarray(t.reshape(B, C, H // 2, W // 2))
    result = (rs(out["ll"]), (rs(out["lh"]), rs(out["hl"]), rs(out["hh"])))
    if _trace:
        return result, res
    return result


# revision 13
# speedup vs baseline: 1.2883x; 1.0063x over previous
"""Haar DWT (db1, zero-padded left/top, stride 2) on 8 Trainium2 NeuronCores.

Math (per image, per output index h, w in [0, 512)):
  with A=x[2h-1,2w-1], B=x[2h-1,2w], C=x[2h,2w-1], D=x[2h,2w]  (x[-1,*]=x[*,-1]=0)
  LL = .5(A+B+C+D)   LH = .5(C+D-A-B)   HL = .5(B-A+D-C)   HH = .5(A-B-C+D)

Kernel strategy per core (6 of the 48 (b,c) images, pure data parallel):
  - x rows tiled into 128-row SBUF tiles; tile 0 holds rows 0..127, tile t>0
    holds rows 128t-1..128t+126, so vertical Haar pairs are adjacent
    partition pairs inside one tile.
  - Vertical stage on the TensorEngine: a fp32 128x128 weight matrix holds
    both vertical filters (lo -> psum partitions 0:64, hi -> 64:128) with the
    full 0.5 scale folded in; tile 0 uses a variant whose first output row
    reads only x row 0 (row -1 is zero).  psum[p, j] = vertical result at
    x column j.
  - PSUM is evacuated by a ScalarE copy; the horizontal stage is then two
    VectorE ops per tile: pair-sum of columns (2w-1, 2w) -> LL/LH bands,
    pair-diff -> HL/HH.  Column 0 of each band is the vertical result at
    x column 0 (x column -1 is zero) - two tiny ScalarE copies.
  - All four bands land in ONE dram tensor out4[4, n_img, 512, 512] so each
    per-image band-pair tile drains with a single 128-partition 2 MiB DMA.
"""

import numpy as np

_H = 1024
_W = 1024
_N_CORES = 8


def _build_bass(n_img: int, H: int, W: int):
    import concourse.bass as bass
    import concourse.tile as tile
    from concourse import bacc, mybir

    f32 = mybir.dt.float32
    Ho, Wo = H // 2, W // 2
    n_t = H // 128  # 128-row tiles per image

    # Bacc (not raw Bass): its compile() legalizes sync waits — TRN2 allows
    # at most 1 wait per instruction and walrus rejects the raw Tile output
    nc = bacc.Bacc("TRN2", target_bir_lowering=False, debug=False)
    x = nc.declare_dram_parameter("x", [n_img, H, W], f32, isOutput=False)
    wv = nc.declare_dram_parameter("wv", [2, 128, 128], f32, isOutput=False)
    # [P/M, img, row-tile, lo/hi, p, w]: (lo/hi, p) merges into one
    # 128-partition DMA dim, so each band-pair tile drains in ONE 3-dim DMA
    out2 = nc.declare_dram_parameter(
        "out2", [2, n_img, n_t, 2, 64, Wo], f32, isOutput=True
    )

    # matmul width chunks (fp32 moving operand max 512)
    chunks = [(c, min(512, W - c)) for c in range(0, W, 512)]

    n_mid = 4
    # input DMA chunks: row-tile 0 alone (small, unblocks the first matmuls
    # fast), then the rest in two pieces for finer-grained prefetch
    if n_t > 2:
        in_chunks = [(1, n_t // 2), (n_t // 2, n_t)]
    else:
        in_chunks = [(1, n_t)]
    with tile.TileContext(nc) as tc:
        with (
            tc.tile_pool(name="wpool", bufs=1) as wpool,
            tc.tile_pool(name="xin", bufs=3) as xin,
            tc.tile_pool(name="ps", bufs=4, space=bass.MemorySpace.PSUM) as pspool,
            tc.tile_pool(name="mid", bufs=1) as midpool,
            tc.tile_pool(name="band", bufs=2) as bandpool,
        ):
            wt = wpool.tile([128, 256], f32)
            # wv[0] = first-row-tile weights, wv[1] = standard weights; loaded
            # via SWDGE so the HWDGE ring starts on image-0 data immediately
            nc.gpsimd.dma_start(
                wt.rearrange("k (two m) -> k two m", two=2),
                wv.rearrange("two k m -> k two m"),
            )

            # persistent psum-evac tiles; col 1 is a zero pad (x col -1), data
            # starts at col 2 (8B-aligned so the ScalarE copy keeps 2x mode)
            mids = []
            for k in range(n_mid):
                m = midpool.tile([128, W + 8], f32, tag=f"mid{k}")
                nc.gpsimd.memset(m[:, 0:2], 0.0)
                mids.append(m)

            for i in range(n_img):
                bp = bandpool.tile([128, n_t * Wo], f32, tag="bp")
                bm = bandpool.tile([128, n_t * Wo], f32, tag="bm")
                # row tile 0 = x rows 0..127; row tile t>0 = rows 128t-1..+126
                xt = xin.tile([128, n_t * W], f32)
                nc.sync.dma_start(xt[:, 0:W], x[i, 0:128, :])
                for ta, tb in in_chunks:
                    nc.sync.dma_start(
                        xt[:, ta * W : tb * W].rearrange("p (t w) -> p t w", t=tb - ta),
                        x[i, 128 * ta - 1 : 128 * tb - 1, :].rearrange(
                            "(t p) w -> p t w", p=128
                        ),
                    )
                for t in range(n_t):
                    w_ap = wt[:, 0:128] if t == 0 else wt[:, 128:256]
                    ps = pspool.tile([128, W], f32)
                    for c, cw in chunks:
                        nc.tensor.matmul(
                            ps[:, c : c + cw],
                            w_ap,
                            xt[:, t * W + c : t * W + c + cw],
                            start=True,
                            stop=True,
                        )

                    # evacuate PSUM next to the zero pad: mid col j+2 = x col j
                    mid = mids[(i * n_t + t) % n_mid]
                    nc.scalar.copy(mid[:, 2 : W + 2], ps[:])

                    # horizontal pairs: out w <- mid cols (2w+1, 2w+2)
                    o = t * Wo
                    nc.vector.tensor_add(
                        bp[:, o : o + Wo], mid[:, 1 : W + 1 : 2], mid[:, 2 : W + 2 : 2]
                    )
                    nc.vector.tensor_sub(
                        bm[:, o : o + Wo], mid[:, 2 : W + 2 : 2], mid[:, 1 : W + 1 : 2]
                    )

                    # drain half-image chunks on the SWDGE ring: keeping stores
                    # off the HWDGE ring lets next-image loads issue immediately.
                    # The final image drains in quarters so the kernel tail only
                    # waits on a small last store.
                    last_img = i == n_img - 1
                    dn = max(n_t // (4 if last_img and n_t >= 4 else 2), 1)
                    if t % dn == dn - 1:
                        h0 = t + 1 - dn
                        sl = slice(h0 * Wo, (t + 1) * Wo)
                        nc.gpsimd.dma_start(
                            out2[0, i, h0 : t + 1].rearrange("t b p w -> (b p) t w"),
                            bp[:, sl].rearrange("pp (t w) -> pp t w", t=dn),
                        )
                        nc.gpsimd.dma_start(
                            out2[1, i, h0 : t + 1].rearrange("t b p w -> (b p) t w"),
                            bm[:, sl].rearrange("pp (t w) -> pp t w", t=dn),
                        )
    # run Bacc's legalization passes (walrus needs <=1 sync wait per inst);
    # run_bass_via_pjrt serializes the module as-is
    nc.finalize()
    return nc


def _wv_matrices(h0_v, h1_v, hscale):
    """[2,128,128]: [0] = first-row-tile weights (pairs shifted, row -1 is
    zero), [1] = standard weights for tiles starting at row 128t-1."""
    wv = np.zeros((2, 128, 128), np.float32)
    j = np.arange(64)
    # standard: tile partition k holds x row 128t-1+k; out j pairs k=(2j, 2j+1)
    wv[1, 2 * j, j] = hscale * h0_v[0]
    wv[1, 2 * j + 1, j] = hscale * h0_v[1]
    wv[1, 2 * j, 64 + j] = hscale * h1_v[0]
    wv[1, 2 * j + 1, 64 + j] = hscale * h1_v[1]
    # tile 0: partition k holds x row k; out j pairs k=(2j-1, 2j)
    wv[0, 0, 0] = hscale * h0_v[1]
    wv[0, 0, 64] = hscale * h1_v[1]
    j = np.arange(1, 64)
    wv[0, 2 * j - 1, j] = hscale * h0_v[0]
    wv[0, 2 * j, j] = hscale * h0_v[1]
    wv[0, 2 * j - 1, 64 + j] = hscale * h1_v[0]
    wv[0, 2 * j, 64 + j] = hscale * h1_v[1]
    return wv


def _ensure_ntff_hook():
    """Provide antenv.axon_hooks + register the ctypes NTFF profile hook
    (the agent image's antenv lacks axon_hooks, so tracing would crash)."""
    import contextlib
    import ctypes
    import sys
    import types

    if "antenv.axon_hooks" not in sys.modules:
        mod = types.ModuleType("antenv.axon_hooks")
        state = {"hook": None}
        mod.set_axon_ntff_profile_hook = lambda h: state.__setitem__("hook", h)
        mod.get_axon_ntff_profile_hook = lambda: state["hook"]
        sys.modules["antenv.axon_hooks"] = mod
        try:
            import antenv

            antenv.axon_hooks = mod
        except ImportError:
            pass
    hooks = sys.modules["antenv.axon_hooks"]
    if hooks.get_axon_ntff_profile_hook() is not None:
        return

    lib = ctypes.CDLL("/opt/axon/libaxon_pjrt.so")
    if not hasattr(lib, "axon_start_nrt_profile"):
        return
    lib.axon_start_nrt_profile.argtypes = [
        ctypes.POINTER(ctypes.c_int64),
        ctypes.c_size_t,
    ]
    lib.axon_start_nrt_profile.restype = ctypes.c_int64
    lib.axon_stop_nrt_profile.argtypes = [ctypes.c_char_p]
    lib.axon_stop_nrt_profile.restype = ctypes.c_int64

    @contextlib.contextmanager
    def _hook(output_dir, device_ids):
        import jax

        jax.devices()
        if device_ids:
            ids = (ctypes.c_int64 * len(device_ids))(*device_ids)
            rc = lib.axon_start_nrt_profile(ids, len(device_ids))
        else:
            rc = lib.axon_start_nrt_profile(None, 0)
        if rc != 0:
            raise RuntimeError(f"axon_start_nrt_profile rc={rc}")
        try:
            yield
        finally:
            n = lib.axon_stop_nrt_profile(str(output_dir).encode())
            print(f"profile: {n} file(s) written to {output_dir}", file=sys.stderr)

    hooks.set_axon_ntff_profile_hook(_hook)

    # the post-run artifact upload needs bucket creds we don't have
    import concourse.bass_utils as bu

    bu.upload_artifacts = lambda tmpdir: f"local://{tmpdir}"


def _run(x, wv, trace=False, tmpdir=None):
    """x: (48, 1024, 1024) f32. Returns dict band -> (48, 512, 512), results."""
    import sys

    if "/opt/trn_rl_repo" not in sys.path:
        sys.path.insert(0, "/opt/trn_rl_repo")
    if trace:
        _ensure_ntff_hook()
    from concourse.bass_utils import run_bass_kernel_spmd

    n_total = x.shape[0]
    per = n_total // _N_CORES
    nc = _build_bass(per, _H, _W)
    in_maps = [
        {"x": np.ascontiguousarray(x[c * per : (c + 1) * per]), "wv": wv}
        for c in range(_N_CORES)
    ]
    res = run_bass_kernel_spmd(
        nc, in_maps, list(range(_N_CORES)), trace=trace, tmpdir=tmpdir
    )
    # out2: (2, per, n_t, 2, 64, Wo); h = 64*t + p
    parts = [res.results[c]["out2"] for c in range(_N_CORES)]
    full = np.concatenate(parts, axis=1)  # (2, 48, n_t, 2, 64, Wo)
    n_total_, nt_, _, _, wo_ = full.shape[1:]
    banded = full.transpose(0, 3, 1, 2, 4, 5).reshape(2, 2, n_total_, nt_ * 64, wo_)
    return {
        "ll": banded[0, 0],
        "lh": banded[0, 1],
        "hl": banded[1, 0],
        "hh": banded[1, 1],
    }, res


def kernel(x, h0_h, h1_h, h0_v, h1_v, _trace=False, _tmpdir=None):
    x = np.asarray(x, dtype=np.float32)
    h0_h = np.asarray(h0_h, dtype=np.float32).reshape(-1)
    h1_h = np.asarray(h1_h, dtype=np.float32).reshape(-1)
    h0_v = np.asarray(h0_v, dtype=np.float32).reshape(-1)
    h1_v = np.asarray(h1_v, dtype=np.float32).reshape(-1)

    # the horizontal stage is a plain pair sum/diff; that requires the
    # Haar shape [s, s] / [-s, s] (always the case for this problem)
    hs = float(h0_h[1])
    assert abs(h0_h[0] - hs) < 1e-6 and abs(h1_h[1] - hs) < 1e-6
    assert abs(h1_h[0] + hs) < 1e-6

    B, C, H, W = x.shape
    xf = x.reshape(B * C, H, W)
    wv = _wv_matrices(h0_v, h1_v, hs)
    out, res = _run(xf, wv, trace=_trace, tmpdir=_tmpdir)
    rs = lambda t: np.ascontiguousarray(t.reshape(B, C, H // 2, W // 2))
    result = (rs(out["ll"]), (rs(out["lh"]), rs(out["hl"]), rs(out["hh"])))
    if _trace:
        return result, res
    return result
